# revision 53
# baseline (speedup 1.0000x reference)
"""ArcticDecoderLayer on 8 TRN2 NeuronCores.

Sharding (expert-parallel per the hint):
 - MoE: core c owns expert c. Host computes top-2 routing and DISPATCHES:
   core c gets only the tokens routed to expert c (zero-padded to capacity
   C), computes silu(xd @ w1_c) * (xd @ w3_c) * w_routing, then @ w2_c.
   Host scatter-adds the per-core outputs back into the sequence.
 - Dense residual MLP: column-sharded across cores; partials summed on
   the host with the MoE partials.
 - Attention / norms / gate are tiny (<12% of layer FLOPs) and run on the
   host as input prep.

Device kernel details (hand-rolled, one shared 8-bank PSUM pool so phase
boundaries pipeline instead of hitting pool-teardown WAR barriers):
 - Both MoE projections run in fp8(e4m3) with DoubleRow (double-pumped
   PE, ~149 TF/s measured); fp32 scales fold into the silu input scale
   and the routing-weight vector. The residual MLP stays bf16 — fp8
   there busts the 2e-2 error budget (measured 1.864e-2 total).
 - Exact capacity-C (288) matmul streams; silu-gating reads straight
   out of PSUM; gated hidden states live entirely in SBUF.
 - DMA-ring discipline: loads on the sync HWDGE ring as ONE large
   strided dma_start per weight tile (descriptor generation costs
   ~0.6us of sequencer time per dma_start regardless of size); stores
   ride the scalar HWDGE ring (res2, batched per group) and the sync
   ring (mm2, after all loads) so they never head-of-line block loads;
   xall/wb/rw2 ride the otherwise-idle GPSIMD SWDGE ring.
 - Head: the sync ring only reaches full rate ~10us in, so res1's kt0
   block (1.5MB in, 7us of PE work) front-runs mm1 in 128KB chunks
   behind ~3us of HAM warm-up matmuls; its partials park in SBUF and
   are added back during res1's gating. mm1's first tiles stream
   per-kt so no PE stall exceeds the 3.4us HAM re-throttle window.
 - All weight/activation inputs are PRE-TILED on the host into the
   exact [k_tile][m_tile][128, ksub*mtile] blocks the matmuls consume,
   so every DMA is 128 long contiguous per-partition lines.
"""

from contextlib import ExitStack

import ml_dtypes
import numpy as np

from concourse import bacc, mybir, tile
import concourse.bass as bass
from concourse.bass import ds, ts
from concourse.bass_utils import run_bass_kernel_spmd
from concourse.kernels.tile_matmul import (
    ShapeInfo,
    composable_matmul_tile_kernel,
    dma_to_dram_mxn,
)

B, S, H = 1, 1024, 2048
NH, HD, KVH = 32, 64, 8
E, F, TOPK = 8, 2048, 2
EPS = 1e-6
ROPE_THETA = 10000.0
N_CORES = 8
FSH = F // N_CORES  # res-mlp ffn shard = 256
BF16 = ml_dtypes.bfloat16
FP8 = ml_dtypes.float8_e4m3  # TRN FP8_EXP4 (max normal +-240)

FP8_UP = True     # MoE up-projection in fp8 (double-pumped PE)
FP8_DOWN = True   # MoE down-projection in fp8 (double-pumped PE)
SX, SW, SH_ = 16.0, 64.0, 16.0  # fp8 scales: activations / weights / hidden

LAST_RESULTS = None  # stashed BassKernelResults for test harnesses

_COMPILED = {}


def _tiled_dram_producer(nc, pool, dram_t, ksub, mtile, dtype, tag, idx_attr,
                         memo=None):
    """kxm/kxn producer reading pre-tiled [KT, MT, 128, ksub*mtile] blocks.

    memo: optional dict — lets first tiles be pre-fetched early in the
    program (hoisted into an earlier phase's DMA stream) and returned
    again when the matmul kernel asks for them.
    """
    def producer(nc_, md):
        key = (md.k_tile_idx, getattr(md, idx_attr))
        if memo is not None and key in memo:
            return memo.pop(key)
        t = pool.tile([128, ksub, mtile], dtype, tag=tag)
        src = dram_t[md.k_tile_idx, getattr(md, idx_attr)]
        nc_.sync.dma_start(
            out=t[:], in_=src.rearrange("p (j c) -> p j c", j=ksub))
        if memo is not None:
            memo[key] = t
        return t
    return producer


def _build_nc(C, fp8_up, fp8_down):
    """C = per-expert token capacity (multiple of 32)."""
    nc = bacc.Bacc("TRN2", target_bir_lowering=False, debug=False,
                   num_devices=N_CORES)
    f32 = mybir.dt.float32
    bf16 = mybir.dt.bfloat16
    fp8 = mybir.dt.float8e4
    updt = fp8 if fp8_up else bf16
    hdt = fp8 if fp8_down else bf16

    # pre-tiled inputs: [K_TILES, M_TILES, 128, ksub*mtile]
    ew13 = nc.dram_tensor("ew13", [4, 8, 128, 2048], updt, kind="ExternalInput")
    xdT = nc.dram_tensor("xdT", [4, 1, 128, 4 * C], updt, kind="ExternalInput")
    ew2 = nc.dram_tensor("ew2", [4, 4, 128, 2048], hdt, kind="ExternalInput")
    rw13 = nc.dram_tensor("rw13", [4, 1, 128, 2048], bf16, kind="ExternalInput")
    hrT = nc.dram_tensor("hrT", [4, 2, 128, 2048], bf16, kind="ExternalInput")
    rw2 = nc.dram_tensor("rw2", [1, 4, 128, 1024], bf16, kind="ExternalInput")
    wvec = nc.dram_tensor("wvec", [1, C], f32, kind="ExternalInput")
    moe_out = nc.dram_tensor("moe_out", [H, C], bf16, kind="ExternalOutput")
    res_out = nc.dram_tensor("res_out", [S, H], bf16, kind="ExternalOutput")

    silu_scale = 1.0 / (SX * SW) if fp8_up else 1.0

    with tile.TileContext(nc) as tc:
        with tc.tile_pool(name="persist", bufs=1) as ppool, \
             tc.tile_pool(name="gate", bufs=3) as gpool, \
             ExitStack() as stack:
            # SBUF-resident gated activations
            hT_sb = ppool.tile([128, 16, C], hdt, tag="hT_sb")
            hrs_sb = ppool.tile([128, 2, S], bf16, tag="hrs_sb")

            # Warm-up source tile for the PE HAM clock-gate (the dummy
            # matmuls themselves are issued inside the mm1 PSUM pool so
            # the first real matmul isn't serialized behind a WAR barrier
            # on a dead warm-up PSUM bank).
            warm = ppool.tile([128, 256], bf16, tag="warm")
            nc.vector.memset(warm[:], 0.0)

            def pool_(name, bufs):
                return stack.enter_context(tc.tile_pool(name=name, bufs=bufs))

            # All pools are created up front, before any matmul phase, so
            # no phase's first DMAs are blocked behind a WAR barrier on
            # SBUF address ranges recycled from the previous phase.
            kxm1pool = pool_("kxm1", 4)
            rw13pool = pool_("rw13p", 4)
            hrTpool = pool_("hrTp", 4)
            ew2pool = pool_("ew2p", 4)
            rw2pool = pool_("rw2p", 1)
            prod_pool = pool_("prod", 4)
            # ONE PSUM pool for every phase: all matmul groups cycle the
            # same 4 tags x 2 bufs, so phase boundaries are ordinary
            # pipeline waits instead of pool-teardown WAR barriers.
            pspool = stack.enter_context(
                tc.tile_pool(name="ps", bufs=2, space="PSUM"))

            def psum(mi):
                return pspool.tile([128, 512], f32, tag=f"mps{mi}",
                                   name=f"mps{mi}")

            def load_tile(pool, tag, dram_t, dtype=bf16, j=4):
                t = pool.tile([128, j, dram_t.shape[-1] // j], dtype, tag=tag)
                nc.sync.dma_start(
                    out=t[:], in_=dram_t.rearrange("p (j c) -> p j c", j=j))
                return t

            # ---- mm1: MoE up-proj, hand-rolled for exact-C streams ----
            # host packs ew13 as [w1_b0|w3_b0|w1_b1|w3_b1|...] (256-col
            # blocks): m-tile m holds subtiles [w1a, w1b, w3a, w3b] for
            # f-rows m*256..(m+1)*256. Gating reads straight from PSUM.
            # Warm the PE HAM clock-gate during the initial DMA wait: the
            # gate needs ~3.4us of sustained activity to lift the PE from
            # 1.2 to 2.4 GHz; dummy matmuls bridge the gap until the
            # first weight chunks land.
            # 16 N=256 dummies span ~3.4us — exactly the sustained-активity
            # window HAM needs — so the PE hits 2.4 GHz right as the first
            # real operands land.
            # 14 N=256 dummies span ~3us of sustained PE activity, so the
            # HAM clock-gate lifts to 2.4 GHz right as res1-kt0's first
            # chunks land (~10.5us) and the front-run work runs warm.
            wps = psum(0)
            for _ in range(12):
                nc.tensor.matmul(wps[:, :256], warm[:, :128], warm[:],
                                 start=True, stop=True)
            # ---- head: res1's kt0 block front-runs mm1 ----
            # The sync ring only ramps to full rate ~10us in, and mm1
            # needs 241 GB/s sustained — so the first ~7us of real PE work
            # is res1's kt0 accumulation block, whose inputs are only
            # 1.5MB and stream in 128KB chunks right behind the ramp.
            # Its partial sums are evicted to SBUF and added back during
            # res1's gating.
            rwt0 = rw13pool.tile([128, 4, 512], bf16, tag="t_rw13")
            ht0 = hrTpool.tile([128, 4, 512], bf16, tag="t_hrT")
            for ks in range(4):
                nc.sync.dma_start(
                    out=rwt0[:, ks:ks + 1, :].rearrange("p a c -> p (a c)"),
                    in_=rw13[0, 0][:, 512 * ks:512 * (ks + 1)])
                nc.sync.dma_start(
                    out=ht0[:, ks:ks + 1, :].rearrange("p a c -> p (a c)"),
                    in_=hrT[0, 0][:, 512 * ks:512 * (ks + 1)])
            # xall + small persistents ride the GPSIMD (SWDGE) ring
            xall = ppool.tile([128, 16, C], updt, tag="xall")
            nc.gpsimd.dma_start(
                out=xall[:, 0:4, :],
                in_=xdT[0, 0].rearrange("p (j c) -> p j c", j=4))
            nc.gpsimd.dma_start(
                out=xall[:, 4:16, :].rearrange("p (k j) c -> p k (j c)", k=3),
                in_=xdT[1:4, 0].rearrange("k p f -> p k f"))
            wv_ap = wvec[:]
            wb = ppool.tile([128, C], f32, tag="wb")
            bcast = bass.AP(tensor=wv_ap.tensor, offset=wv_ap.offset,
                            ap=[[0, 128], wv_ap.ap[-1]])
            nc.gpsimd.dma_start(out=wb[:], in_=bcast)
            wb_u = wb[:].unsqueeze(1)
            rw2_t = rw2pool.tile([128, 8, 512], bf16, tag="t_rw2")
            nc.gpsimd.dma_start(
                out=rw2_t[:].rearrange("p (n j) c -> p n (j c)", n=4),
                in_=rw2[0, :].rearrange("n p f -> p n f"))

            # res1-kt0-n0: 16 N=512 matmuls, ks-ordered to follow the
            # chunk arrivals (1MB total matches the ring's ramp rate —
            # front-running both n-tiles needed 218 GB/s and stalled);
            # partials evicted to SBUF, folded back at res1's gating.
            part_sb = ppool.tile([128, 4, 512], f32, tag="part")
            ps_r0 = [psum(mi) for mi in range(4)]
            for ks in range(4):
                for mi in range(4):
                    nc.tensor.matmul(
                        ps_r0[mi][:, :512],
                        rwt0[:, ks:ks + 1, 128 * mi:128 * (mi + 1)],
                        ht0[:, ks:ks + 1, :],
                        start=(ks == 0), stop=(ks == 3))
            for mi in range(4):
                if mi % 2 == 0:
                    nc.vector.tensor_copy(out=part_sb[:, mi, :],
                                          in_=ps_r0[mi][:, :512])
                else:
                    nc.scalar.copy(out=part_sb[:, mi, :],
                                   in_=ps_r0[mi][:, :512])

            # ---- mm1: MoE up-proj, fp8 DoubleRow ----
            kstep = 2 if fp8_up else 1
            pm = mybir.MatmulPerfMode.DoubleRow if fp8_up else None
            for m in range(8):
                if m <= 1:
                    # per-kt chunks: the ring is just past its ramp
                    t = kxm1pool.tile([128, 16, 512], updt, tag="t_ew13")
                    for kt in range(4):
                        nc.sync.dma_start(
                            out=t[:, 4 * kt:4 * kt + 4, :].rearrange(
                                "p j c -> p (j c)"),
                            in_=ew13[kt, m])
                else:
                    t = kxm1pool.tile([128, 16, 512], updt, tag="t_ew13")
                    nc.sync.dma_start(
                        out=t[:].rearrange("p (k j) c -> p k (j c)", k=4),
                        in_=ew13[:, m].rearrange("k p f -> p k f"))
                ps = [psum(mi) for mi in range(4)]
                for kt in range(4):
                    for mi in range(4):
                        for kp in range(0, 4, kstep):
                            nc.tensor.matmul(
                                ps[mi][:, :C],
                                t[:, 4 * kt + kp:4 * kt + kp + kstep,
                                  128 * mi:128 * (mi + 1)],
                                xall[:, 4 * kt + kp:4 * kt + kp + kstep, :],
                                start=(kt == 0 and kp == 0),
                                stop=(kt == 3 and kp + kstep == 4),
                                perf_mode=pm)
                for half in range(2):
                    sm = gpool.tile([128, 1, C], f32, tag="sm")
                    nc.scalar.activation(
                        sm[:], ps[half][:, :C].unsqueeze(1),
                        mybir.ActivationFunctionType.Silu,
                        scale=silu_scale)
                    nc.vector.tensor_mul(
                        sm[:], sm[:], ps[half + 2][:, :C].unsqueeze(1))
                    nc.vector.tensor_mul(
                        hT_sb[:, 2 * m + half:2 * m + half + 1, :C],
                        sm[:], wb_u)

            # ---- res1 rest: kt0 (n1 only) + kt1-3 both n, K-OUTER ----
            # n0 accumulates kt1-3 and adds the front-run partial at
            # gating; n1 accumulates all four kt here.  Both n-tiles'
            # accumulators live in PSUM at once (8 banks).
            # allocate n1's accumulators FIRST: they cycle onto the PSUM
            # buffers m6's gating freed (not m7's), so the kt0-n1 block
            # that opens res1 doesn't wait on m7's gating chain.
            ps_r = [None, [psum(mi) for mi in range(4)]]
            ps_r[0] = [psum(mi) for mi in range(4)]
            ht0n1 = hrTpool.tile([128, 4, 512], bf16, tag="t_hrT")
            nc.sync.dma_start(
                out=ht0n1[:],
                in_=hrT[0, 1].rearrange("p (j c) -> p j c", j=4))
            for mi in range(4):
                for ks in range(4):
                    nc.tensor.matmul(
                        ps_r[1][mi][:, :512],
                        rwt0[:, ks:ks + 1, 128 * mi:128 * (mi + 1)],
                        ht0n1[:, ks:ks + 1, :],
                        start=(ks == 0), stop=False)
            for kt in range(1, 4):
                rwt = rw13pool.tile([128, 4, 512], bf16, tag="t_rw13")
                nc.sync.dma_start(
                    out=rwt[:],
                    in_=rw13[kt, 0].rearrange("p (j c) -> p j c", j=4))
                ht = hrTpool.tile([128, 8, 512], bf16, tag="t_hrT")
                nc.sync.dma_start(
                    out=ht[:].rearrange("p (n j) c -> p n (j c)", n=2),
                    in_=hrT[kt, :].rearrange("n p f -> p n f"))
                for n in range(2):
                    for mi in range(4):
                        for ks in range(4):
                            nc.tensor.matmul(
                                ps_r[n][mi][:, :512],
                                rwt[:, ks:ks + 1,
                                    128 * mi:128 * (mi + 1)],
                                ht[:, 4 * n + ks:4 * n + ks + 1, :],
                                start=(n == 0 and kt == 1 and ks == 0),
                                stop=(kt == 3 and ks == 3))
            for n in range(2):
                for half in range(2):
                    if n == 0:
                        u1f = gpool.tile([128, 1, 512], f32, tag="u1f")
                        nc.vector.tensor_add(
                            u1f[:], ps_r[0][half][:, :512].unsqueeze(1),
                            part_sb[:, half:half + 1, :])
                        u3f = gpool.tile([128, 1, 512], f32, tag="u3f")
                        nc.vector.tensor_add(
                            u3f[:], ps_r[0][2 + half][:, :512].unsqueeze(1),
                            part_sb[:, 2 + half:3 + half, :])
                    else:
                        u1f = ps_r[1][half][:, :512].unsqueeze(1)
                        u3f = ps_r[1][2 + half][:, :512].unsqueeze(1)
                    smr = gpool.tile([128, 1, 512], f32, tag="smr")
                    nc.scalar.activation(
                        smr[:], u1f[:] if n == 0 else u1f,
                        mybir.ActivationFunctionType.Silu)
                    nc.vector.tensor_mul(
                        hrs_sb[:, half:half + 1, n * 512:(n + 1) * 512],
                        smr[:], u3f[:] if n == 0 else u3f)

            # ---- res2: residual-MLP down-proj (kxm = SBUF hrs) ----
            # res_out stores go out on the ACT (scalar) HWDGE ring so
            # they never head-of-line block the weight loads streaming on
            # the sync ring.
            res_out_ap = res_out[:].rearrange("(mt ms p) n -> p mt ms n",
                                              p=128, ms=4)
            for m in range(2):
                for nn in range(4):
                    mo = prod_pool.tile([128, 4, 512], bf16, tag="prodr")
                    for mi in range(4):
                        ps2 = psum(mi)
                        for ks in range(2):
                            nc.tensor.matmul(
                                ps2[:, :512],
                                hrs_sb[:, ks,
                                       m * 512 + 128 * mi:
                                       m * 512 + 128 * (mi + 1)],
                                rw2_t[:, 2 * nn + ks:2 * nn + ks + 1, :],
                                start=(ks == 0), stop=(ks == 1))
                        # alternate eviction engine: vector alone can't
                        # quite keep up with res2's 8-MM accumulation
                        # groups, which was pacing the PE
                        if mi % 2 == 0:
                            nc.vector.tensor_copy(out=mo[:, mi, :],
                                                  in_=ps2[:, :512])
                        else:
                            nc.scalar.copy(out=mo[:, mi, :],
                                           in_=ps2[:, :512])
                    # one batched store per group: each dma_start costs
                    # ~0.6us of descriptor generation on its sequencer,
                    # so 8 big stores beat 32 small ones by ~14us of
                    # scalar-engine time.
                    nc.scalar.dma_start(
                        out=res_out_ap[:, m, :, nn * 512:(nn + 1) * 512],
                        in_=mo[:, :, :])

            # ---- mm2: MoE down-proj, hand-rolled with ew2 stationary ----
            # moe_outT[h, t] = sum_f ew2[f, h] * hT[f, t]: streaming the
            # C real tokens; output lands transposed ([H, C]) which the
            # host scatter handles for free. The 16-deep ew2 pool lets the
            # whole of ew2 prefetch during res1/res2 on the sync ring.
            kstep2 = 2 if fp8_down else 1
            pm2 = mybir.MatmulPerfMode.DoubleRow if fp8_down else None
            for mt in range(4):
                t2 = ew2pool.tile([128, 16, 512], hdt, tag="t_ew2")
                nc.sync.dma_start(
                    out=t2[:].rearrange("p (k j) c -> p k (j c)", k=4),
                    in_=ew2[:, mt].rearrange("k p f -> p k f"))
                # mi-major: each 128-row output strip finishes every ~1us,
                # so its PSUM eviction (vector) + store (scalar ring)
                # pipeline under the remaining matmuls instead of bunching
                # up in the kernel tail.
                mo = prod_pool.tile([128, 4, 512], bf16, tag="prodh")
                for mi in range(4):
                    ps2 = psum(mi)
                    for kt in range(4):
                        for ks in range(0, 4, kstep2):
                            nc.tensor.matmul(
                                ps2[:, :C],
                                t2[:, 4 * kt + ks:4 * kt + ks + kstep2,
                                   128 * mi:128 * (mi + 1)],
                                hT_sb[:, 4 * kt + ks:4 * kt + ks + kstep2, :],
                                start=(kt == 0 and ks == 0),
                                stop=(kt == 3 and ks + kstep2 == 4),
                                perf_mode=pm2)
                    nc.vector.tensor_copy(out=mo[:, mi, :C],
                                          in_=ps2[:, :C])
                    dst = moe_out[mt * 512 + mi * 128:
                                  mt * 512 + (mi + 1) * 128, :]
                    # stores ride the sync ring, which is idle by now
                    # (all loads issued) — the scalar ring is still
                    # draining res2's stores.
                    nc.sync.dma_start(
                        out=dst.rearrange("(o p) c -> p o c", p=128),
                        in_=mo[:, mi:mi + 1, :C])

    nc.compile()
    return nc


def _np_softmax(x, axis=-1):
    m = np.max(x, axis=axis, keepdims=True)
    e = np.exp(x - m)
    return e / np.sum(e, axis=axis, keepdims=True)


def _rmsnorm(x, w):
    v = np.mean(np.square(x), axis=-1, keepdims=True)
    return x / np.sqrt(v + EPS) * w


def _tile_pack(W, k_tile, m_tile):
    """[K, M] -> [KT, MT, 128, ksub*m_tile] matching the device producers."""
    K, M = W.shape
    kt, mt, ks = K // k_tile, M // m_tile, k_tile // 128
    return np.ascontiguousarray(
        W.reshape(kt, ks, 128, mt, m_tile)
        .transpose(0, 3, 2, 1, 4)
        .reshape(kt, mt, 128, ks * m_tile))


def kernel(hidden_states, attention_mask, position_ids, wq, wk, wv, wo,
           norm1_w, norm_res_w, res_w1, res_w3, res_w2,
           gate_w, e_w1, e_w3, e_w2):
    global LAST_RESULTS
    f4 = np.float32
    x = np.asarray(hidden_states, f4).reshape(S, H)
    amask = np.asarray(attention_mask).reshape(S)
    pos = np.asarray(position_ids).reshape(S).astype(np.int64)

    # ---- host: attention (tiny vs MoE) ----
    inv_freq = 1.0 / (ROPE_THETA ** (np.arange(0, HD, 2, dtype=f4) / HD))
    t = np.arange(S, dtype=f4)
    freqs = np.outer(t, inv_freq)
    emb = np.concatenate([freqs, freqs], axis=-1)
    sin_t, cos_t = np.sin(emb), np.cos(emb)
    s_ = sin_t[pos].astype(f4)
    c_ = cos_t[pos].astype(f4)

    h = _rmsnorm(x, np.asarray(norm1_w, f4))
    q = (h @ np.asarray(wq, f4)).reshape(S, NH, HD).transpose(1, 0, 2)
    k = (h @ np.asarray(wk, f4)).reshape(S, KVH, HD).transpose(1, 0, 2)
    v = (h @ np.asarray(wv, f4)).reshape(S, KVH, HD).transpose(1, 0, 2)

    def rot(z):
        hh = z.shape[-1] // 2
        return np.concatenate([-z[..., hh:], z[..., :hh]], axis=-1)

    q = q * c_[None] + rot(q) * s_[None]
    k = k * c_[None] + rot(k) * s_[None]
    groups = NH // KVH
    k = np.repeat(k, groups, axis=0)
    v = np.repeat(v, groups, axis=0)
    causal = np.tril(np.ones((S, S), bool))
    mask = causal & (amask > 0)[None, :]
    bias = np.where(mask, f4(0.0), np.finfo(f4).min).astype(f4)
    scores = np.einsum('hqd,hkd->hqk', q, k).astype(f4) * f4(1.0 / np.sqrt(HD))
    scores = scores + bias[None]
    p = _np_softmax(scores, axis=-1).astype(f4)
    attn = np.einsum('hqk,hkd->hqd', p, v).transpose(1, 0, 2).reshape(S, H)
    attn = attn @ np.asarray(wo, f4)
    h1 = x + attn
    hr = _rmsnorm(h1, np.asarray(norm_res_w, f4))

    # ---- host: top-2 routing -> per-expert dispatch ----
    logits = x @ np.asarray(gate_w, f4)
    rw_ = _np_softmax(logits.astype(f4), axis=-1)
    ti = np.argsort(-rw_, axis=-1, kind="stable")[:, :TOPK]
    tw = np.take_along_axis(rw_, ti, axis=-1)
    tw = tw / np.sum(tw, axis=-1, keepdims=True)
    wdense = np.zeros((S, E), f4)
    np.add.at(wdense, (np.arange(S)[:, None], ti), tw)

    idxs = [np.where(wdense[:, c] > 0)[0] for c in range(E)]
    maxc = max(len(ix) for ix in idxs)
    C = max(288, -(-maxc // 32) * 32)   # capacity, multiple of 32

    # ---- device: expert-parallel dispatched MoE + sharded residual MLP ----
    key = (C, FP8_UP, FP8_DOWN)
    if key not in _COMPILED:
        _COMPILED[key] = _build_nc(C, FP8_UP, FP8_DOWN)
    nc = _COMPILED[key]

    def b16(a):
        return np.asarray(a, f4).astype(BF16)

    def b8(a, s):
        return np.clip(np.asarray(a, f4) * s, -240.0, 240.0).astype(FP8)

    def pack13(w1, w3, blk=256):
        # [w1_b0|w3_b0|w1_b1|w3_b1|...] in 256-col blocks
        nb = w1.shape[1] // blk
        cols = []
        for m in range(nb):
            cols.append(w1[:, m * blk:(m + 1) * blk])
            cols.append(w3[:, m * blk:(m + 1) * blk])
        return np.concatenate(cols, axis=1)

    xT = np.asarray(x.T, f4)
    hrT_t = _tile_pack(b16(hr.T), 512, 512)
    e_w1 = np.asarray(e_w1, f4)
    e_w3 = np.asarray(e_w3, f4)
    e_w2 = np.asarray(e_w2, f4)
    res_w1 = np.asarray(res_w1, f4)
    res_w3 = np.asarray(res_w3, f4)
    res_w2 = np.asarray(res_w2, f4)

    in_maps = []
    for c in range(N_CORES):
        cs = slice(c * FSH, (c + 1) * FSH)
        ix = idxs[c]
        xdT = np.zeros((H, C), f4)
        xdT[:, :len(ix)] = xT[:, ix]
        wv_c = np.zeros((1, C), f4)
        wv_c[0, :len(ix)] = wdense[ix, c]
        ew13p = pack13(e_w1[c], e_w3[c])
        if FP8_UP:
            xd_dev = b8(xdT, SX)
            ew13_dev = b8(ew13p, SW)
            wv_c = wv_c / (SX * SW)  # fold up-proj descale into routing wt
        else:
            xd_dev = b16(xdT)
            ew13_dev = b16(ew13p)
        if FP8_DOWN:
            wv_c = wv_c * SH_  # h stored as fp8 * SH_
            ew2_dev = b8(e_w2[c], SW)
        else:
            ew2_dev = b16(e_w2[c])
        in_maps.append({
            "xdT": _tile_pack(xd_dev, 512, C),
            "hrT": hrT_t,
            "ew13": _tile_pack(ew13_dev, 512, 512),
            "ew2": _tile_pack(ew2_dev, 512, 512),
            "rw13": _tile_pack(
                b16(np.concatenate([res_w1[:, cs], res_w3[:, cs]], axis=1)),
                512, 512),
            "rw2": _tile_pack(b16(res_w2[cs, :]), 256, 512),
            "wvec": np.ascontiguousarray(wv_c.astype(f4)),
        })

    res = run_bass_kernel_spmd(nc, in_maps, core_ids=list(range(N_CORES)))
    LAST_RESULTS = res

    moe_descale = 1.0 / (SH_ * SW) if FP8_DOWN else 1.0
    out = h1.copy()
    for c in range(N_CORES):
        ix = idxs[c]
        out[ix] += np.asarray(res.results[c]["moe_out"], f4).T[:len(ix)] \
            * moe_descale
        out += np.asarray(res.results[c]["res_out"], f4)
    return out.reshape(B, S, H).astype(np.float32)



# revision 54
# speedup vs baseline: 1.0055x; 1.0055x over previous
"""ArcticDecoderLayer on 8 TRN2 NeuronCores.

Sharding (expert-parallel per the hint):
 - MoE: core c owns expert c. Host computes top-2 routing and DISPATCHES:
   core c gets only the tokens routed to expert c (zero-padded to capacity
   C), computes silu(xd @ w1_c) * (xd @ w3_c) * w_routing, then @ w2_c.
   Host scatter-adds the per-core outputs back into the sequence.
 - Dense residual MLP: column-sharded across cores; partials summed on
   the host with the MoE partials.
 - Attention / norms / gate are tiny (<12% of layer FLOPs) and run on the
   host as input prep.

Device kernel details (hand-rolled, one shared 8-bank PSUM pool so phase
boundaries pipeline instead of hitting pool-teardown WAR barriers):
 - Both MoE projections run in fp8(e4m3) with DoubleRow (double-pumped
   PE, ~149 TF/s measured); fp32 scales fold into the silu input scale
   and the routing-weight vector. The residual MLP stays bf16 — fp8
   there busts the 2e-2 error budget (measured 1.864e-2 total).
 - Exact capacity-C (288) matmul streams; silu-gating reads straight
   out of PSUM; gated hidden states live entirely in SBUF.
 - DMA-ring discipline: loads on the sync HWDGE ring as ONE large
   strided dma_start per weight tile (descriptor generation costs
   ~0.6us of sequencer time per dma_start regardless of size); stores
   ride the scalar HWDGE ring (res2, batched per group) and the sync
   ring (mm2, after all loads) so they never head-of-line block loads;
   xall/wb/rw2 ride the otherwise-idle GPSIMD SWDGE ring.
 - Head: the sync ring only reaches full rate ~10us in, so res1's kt0
   block (1.5MB in, 7us of PE work) front-runs mm1 in 128KB chunks
   behind ~3us of HAM warm-up matmuls; its partials park in SBUF and
   are added back during res1's gating. mm1's first tiles stream
   per-kt so no PE stall exceeds the 3.4us HAM re-throttle window.
 - All weight/activation inputs are PRE-TILED on the host into the
   exact [k_tile][m_tile][128, ksub*mtile] blocks the matmuls consume,
   so every DMA is 128 long contiguous per-partition lines.
"""

from contextlib import ExitStack

import ml_dtypes
import numpy as np

from concourse import bacc, mybir, tile
import concourse.bass as bass
from concourse.bass import ds, ts
from concourse.bass_utils import run_bass_kernel_spmd
from concourse.kernels.tile_matmul import (
    ShapeInfo,
    composable_matmul_tile_kernel,
    dma_to_dram_mxn,
)

B, S, H = 1, 1024, 2048
NH, HD, KVH = 32, 64, 8
E, F, TOPK = 8, 2048, 2
EPS = 1e-6
ROPE_THETA = 10000.0
N_CORES = 8
FSH = F // N_CORES  # res-mlp ffn shard = 256
BF16 = ml_dtypes.bfloat16
FP8 = ml_dtypes.float8_e4m3  # TRN FP8_EXP4 (max normal +-240)

FP8_UP = True     # MoE up-projection in fp8 (double-pumped PE)
FP8_DOWN = True   # MoE down-projection in fp8 (double-pumped PE)
SX, SW, SH_ = 16.0, 64.0, 16.0  # fp8 scales: activations / weights / hidden

LAST_RESULTS = None  # stashed BassKernelResults for test harnesses

_COMPILED = {}


def _tiled_dram_producer(nc, pool, dram_t, ksub, mtile, dtype, tag, idx_attr,
                         memo=None):
    """kxm/kxn producer reading pre-tiled [KT, MT, 128, ksub*mtile] blocks.

    memo: optional dict — lets first tiles be pre-fetched early in the
    program (hoisted into an earlier phase's DMA stream) and returned
    again when the matmul kernel asks for them.
    """
    def producer(nc_, md):
        key = (md.k_tile_idx, getattr(md, idx_attr))
        if memo is not None and key in memo:
            return memo.pop(key)
        t = pool.tile([128, ksub, mtile], dtype, tag=tag)
        src = dram_t[md.k_tile_idx, getattr(md, idx_attr)]
        nc_.sync.dma_start(
            out=t[:], in_=src.rearrange("p (j c) -> p j c", j=ksub))
        if memo is not None:
            memo[key] = t
        return t
    return producer


def _build_nc(C, NC, fp8_up, fp8_down):
    """C = storage capacity (multiple of 32); NC <= C = matmul free dim.

    DoubleRow only needs the k-plane STRIDE 16-aligned, so tiles stay
    C-padded while the matmuls stream exactly the NC real token slots
    (max per-expert load) — ~5% fewer PE cycles in mm1/mm2.
    """
    nc = bacc.Bacc("TRN2", target_bir_lowering=False, debug=False,
                   num_devices=N_CORES)
    f32 = mybir.dt.float32
    bf16 = mybir.dt.bfloat16
    fp8 = mybir.dt.float8e4
    updt = fp8 if fp8_up else bf16
    hdt = fp8 if fp8_down else bf16

    # pre-tiled inputs: [K_TILES, M_TILES, 128, ksub*mtile]
    ew13 = nc.dram_tensor("ew13", [4, 8, 128, 2048], updt, kind="ExternalInput")
    xdT = nc.dram_tensor("xdT", [4, 1, 128, 4 * C], updt, kind="ExternalInput")
    ew2 = nc.dram_tensor("ew2", [4, 4, 128, 2048], hdt, kind="ExternalInput")
    rw13 = nc.dram_tensor("rw13", [4, 1, 128, 2048], bf16, kind="ExternalInput")
    hrT = nc.dram_tensor("hrT", [4, 2, 128, 2048], bf16, kind="ExternalInput")
    rw2 = nc.dram_tensor("rw2", [1, 4, 128, 1024], bf16, kind="ExternalInput")
    wvec = nc.dram_tensor("wvec", [1, C], f32, kind="ExternalInput")
    moe_out = nc.dram_tensor("moe_out", [H, C], bf16, kind="ExternalOutput")
    res_out = nc.dram_tensor("res_out", [S, H], bf16, kind="ExternalOutput")

    silu_scale = 1.0 / (SX * SW) if fp8_up else 1.0

    with tile.TileContext(nc) as tc:
        with tc.tile_pool(name="persist", bufs=1) as ppool, \
             tc.tile_pool(name="gate", bufs=3) as gpool, \
             ExitStack() as stack:
            # SBUF-resident gated activations
            hT_sb = ppool.tile([128, 16, C], hdt, tag="hT_sb")
            hrs_sb = ppool.tile([128, 2, S], bf16, tag="hrs_sb")

            # Warm-up source tile for the PE HAM clock-gate (the dummy
            # matmuls themselves are issued inside the mm1 PSUM pool so
            # the first real matmul isn't serialized behind a WAR barrier
            # on a dead warm-up PSUM bank).
            warm = ppool.tile([128, 256], bf16, tag="warm")
            nc.vector.memset(warm[:], 0.0)

            def pool_(name, bufs):
                return stack.enter_context(tc.tile_pool(name=name, bufs=bufs))

            # All pools are created up front, before any matmul phase, so
            # no phase's first DMAs are blocked behind a WAR barrier on
            # SBUF address ranges recycled from the previous phase.
            kxm1pool = pool_("kxm1", 4)
            rw13pool = pool_("rw13p", 4)
            hrTpool = pool_("hrTp", 4)
            ew2pool = pool_("ew2p", 4)
            rw2pool = pool_("rw2p", 1)
            prod_pool = pool_("prod", 4)
            # ONE PSUM pool for every phase: all matmul groups cycle the
            # same 4 tags x 2 bufs, so phase boundaries are ordinary
            # pipeline waits instead of pool-teardown WAR barriers.
            pspool = stack.enter_context(
                tc.tile_pool(name="ps", bufs=2, space="PSUM"))

            def psum(mi):
                return pspool.tile([128, 512], f32, tag=f"mps{mi}",
                                   name=f"mps{mi}")

            def load_tile(pool, tag, dram_t, dtype=bf16, j=4):
                t = pool.tile([128, j, dram_t.shape[-1] // j], dtype, tag=tag)
                nc.sync.dma_start(
                    out=t[:], in_=dram_t.rearrange("p (j c) -> p j c", j=j))
                return t

            # ---- mm1: MoE up-proj, hand-rolled for exact-C streams ----
            # host packs ew13 as [w1_b0|w3_b0|w1_b1|w3_b1|...] (256-col
            # blocks): m-tile m holds subtiles [w1a, w1b, w3a, w3b] for
            # f-rows m*256..(m+1)*256. Gating reads straight from PSUM.
            # Warm the PE HAM clock-gate during the initial DMA wait: the
            # gate needs ~3.4us of sustained activity to lift the PE from
            # 1.2 to 2.4 GHz; dummy matmuls bridge the gap until the
            # first weight chunks land.
            # 16 N=256 dummies span ~3.4us — exactly the sustained-активity
            # window HAM needs — so the PE hits 2.4 GHz right as the first
            # real operands land.
            # 14 N=256 dummies span ~3us of sustained PE activity, so the
            # HAM clock-gate lifts to 2.4 GHz right as res1-kt0's first
            # chunks land (~10.5us) and the front-run work runs warm.
            wps = psum(0)
            for _ in range(12):
                nc.tensor.matmul(wps[:, :256], warm[:, :128], warm[:],
                                 start=True, stop=True)
            # ---- head: res1's kt0 block front-runs mm1 ----
            # The sync ring only ramps to full rate ~10us in, and mm1
            # needs 241 GB/s sustained — so the first ~7us of real PE work
            # is res1's kt0 accumulation block, whose inputs are only
            # 1.5MB and stream in 128KB chunks right behind the ramp.
            # Its partial sums are evicted to SBUF and added back during
            # res1's gating.
            rwt0 = rw13pool.tile([128, 4, 512], bf16, tag="t_rw13")
            ht0 = hrTpool.tile([128, 4, 512], bf16, tag="t_hrT")
            for ks in range(4):
                nc.sync.dma_start(
                    out=rwt0[:, ks:ks + 1, :].rearrange("p a c -> p (a c)"),
                    in_=rw13[0, 0][:, 512 * ks:512 * (ks + 1)])
                nc.sync.dma_start(
                    out=ht0[:, ks:ks + 1, :].rearrange("p a c -> p (a c)"),
                    in_=hrT[0, 0][:, 512 * ks:512 * (ks + 1)])
            # xall + small persistents ride the GPSIMD (SWDGE) ring
            xall = ppool.tile([128, 16, C], updt, tag="xall")
            nc.gpsimd.dma_start(
                out=xall[:, 0:4, :],
                in_=xdT[0, 0].rearrange("p (j c) -> p j c", j=4))
            nc.gpsimd.dma_start(
                out=xall[:, 4:16, :].rearrange("p (k j) c -> p k (j c)", k=3),
                in_=xdT[1:4, 0].rearrange("k p f -> p k f"))
            wv_ap = wvec[:]
            wb = ppool.tile([128, C], f32, tag="wb")
            bcast = bass.AP(tensor=wv_ap.tensor, offset=wv_ap.offset,
                            ap=[[0, 128], wv_ap.ap[-1]])
            nc.gpsimd.dma_start(out=wb[:], in_=bcast)
            wb_u = wb[:, :NC].unsqueeze(1)
            rw2_t = rw2pool.tile([128, 8, 512], bf16, tag="t_rw2")
            nc.gpsimd.dma_start(
                out=rw2_t[:].rearrange("p (n j) c -> p n (j c)", n=4),
                in_=rw2[0, :].rearrange("n p f -> p n f"))

            # res1-kt0-n0: 16 N=512 matmuls, ks-ordered to follow the
            # chunk arrivals (1MB total matches the ring's ramp rate —
            # front-running both n-tiles needed 218 GB/s and stalled);
            # partials evicted to SBUF, folded back at res1's gating.
            part_sb = ppool.tile([128, 4, 512], f32, tag="part")
            ps_r0 = [psum(mi) for mi in range(4)]
            for ks in range(4):
                for mi in range(4):
                    nc.tensor.matmul(
                        ps_r0[mi][:, :512],
                        rwt0[:, ks:ks + 1, 128 * mi:128 * (mi + 1)],
                        ht0[:, ks:ks + 1, :],
                        start=(ks == 0), stop=(ks == 3))
            for mi in range(4):
                if mi % 2 == 0:
                    nc.vector.tensor_copy(out=part_sb[:, mi, :],
                                          in_=ps_r0[mi][:, :512])
                else:
                    nc.scalar.copy(out=part_sb[:, mi, :],
                                   in_=ps_r0[mi][:, :512])

            # ---- mm1: MoE up-proj, fp8 DoubleRow ----
            kstep = 2 if fp8_up else 1
            pm = mybir.MatmulPerfMode.DoubleRow if fp8_up else None
            for m in range(8):
                if m <= 1:
                    # per-kt chunks: the ring is just past its ramp
                    t = kxm1pool.tile([128, 16, 512], updt, tag="t_ew13")
                    for kt in range(4):
                        nc.sync.dma_start(
                            out=t[:, 4 * kt:4 * kt + 4, :].rearrange(
                                "p j c -> p (j c)"),
                            in_=ew13[kt, m])
                else:
                    t = kxm1pool.tile([128, 16, 512], updt, tag="t_ew13")
                    nc.sync.dma_start(
                        out=t[:].rearrange("p (k j) c -> p k (j c)", k=4),
                        in_=ew13[:, m].rearrange("k p f -> p k f"))
                ps = [psum(mi) for mi in range(4)]
                for kt in range(4):
                    for mi in range(4):
                        for kp in range(0, 4, kstep):
                            nc.tensor.matmul(
                                ps[mi][:, :NC],
                                t[:, 4 * kt + kp:4 * kt + kp + kstep,
                                  128 * mi:128 * (mi + 1)],
                                xall[:, 4 * kt + kp:4 * kt + kp + kstep,
                                     :NC],
                                start=(kt == 0 and kp == 0),
                                stop=(kt == 3 and kp + kstep == 4),
                                perf_mode=pm)
                for half in range(2):
                    sm = gpool.tile([128, 1, C], f32, tag="sm")
                    nc.scalar.activation(
                        sm[:, :, :NC], ps[half][:, :NC].unsqueeze(1),
                        mybir.ActivationFunctionType.Silu,
                        scale=silu_scale)
                    nc.vector.tensor_mul(
                        sm[:, :, :NC], sm[:, :, :NC],
                        ps[half + 2][:, :NC].unsqueeze(1))
                    nc.vector.tensor_mul(
                        hT_sb[:, 2 * m + half:2 * m + half + 1, :NC],
                        sm[:, :, :NC], wb_u)

            # ---- res1 rest: kt0 (n1 only) + kt1-3 both n, K-OUTER ----
            # n0 accumulates kt1-3 and adds the front-run partial at
            # gating; n1 accumulates all four kt here.  Both n-tiles'
            # accumulators live in PSUM at once (8 banks).
            # allocate n1's accumulators FIRST: they cycle onto the PSUM
            # buffers m6's gating freed (not m7's), so the kt0-n1 block
            # that opens res1 doesn't wait on m7's gating chain.
            ps_r = [None, [psum(mi) for mi in range(4)]]
            ps_r[0] = [psum(mi) for mi in range(4)]
            ht0n1 = hrTpool.tile([128, 4, 512], bf16, tag="t_hrT")
            nc.sync.dma_start(
                out=ht0n1[:],
                in_=hrT[0, 1].rearrange("p (j c) -> p j c", j=4))
            for mi in range(4):
                for ks in range(4):
                    nc.tensor.matmul(
                        ps_r[1][mi][:, :512],
                        rwt0[:, ks:ks + 1, 128 * mi:128 * (mi + 1)],
                        ht0n1[:, ks:ks + 1, :],
                        start=(ks == 0), stop=False)
            for kt in range(1, 4):
                rwt = rw13pool.tile([128, 4, 512], bf16, tag="t_rw13")
                nc.sync.dma_start(
                    out=rwt[:],
                    in_=rw13[kt, 0].rearrange("p (j c) -> p j c", j=4))
                ht = hrTpool.tile([128, 8, 512], bf16, tag="t_hrT")
                nc.sync.dma_start(
                    out=ht[:].rearrange("p (n j) c -> p n (j c)", n=2),
                    in_=hrT[kt, :].rearrange("n p f -> p n f"))
                for n in range(2):
                    for mi in range(4):
                        for ks in range(4):
                            nc.tensor.matmul(
                                ps_r[n][mi][:, :512],
                                rwt[:, ks:ks + 1,
                                    128 * mi:128 * (mi + 1)],
                                ht[:, 4 * n + ks:4 * n + ks + 1, :],
                                start=(n == 0 and kt == 1 and ks == 0),
                                stop=(kt == 3 and ks == 3))
            for n in range(2):
                for half in range(2):
                    if n == 0:
                        u1f = gpool.tile([128, 1, 512], f32, tag="u1f")
                        nc.vector.tensor_add(
                            u1f[:], ps_r[0][half][:, :512].unsqueeze(1),
                            part_sb[:, half:half + 1, :])
                        u3f = gpool.tile([128, 1, 512], f32, tag="u3f")
                        nc.vector.tensor_add(
                            u3f[:], ps_r[0][2 + half][:, :512].unsqueeze(1),
                            part_sb[:, 2 + half:3 + half, :])
                    else:
                        u1f = ps_r[1][half][:, :512].unsqueeze(1)
                        u3f = ps_r[1][2 + half][:, :512].unsqueeze(1)
                    smr = gpool.tile([128, 1, 512], f32, tag="smr")
                    nc.scalar.activation(
                        smr[:], u1f[:] if n == 0 else u1f,
                        mybir.ActivationFunctionType.Silu)
                    nc.vector.tensor_mul(
                        hrs_sb[:, half:half + 1, n * 512:(n + 1) * 512],
                        smr[:], u3f[:] if n == 0 else u3f)

            # ---- res2: residual-MLP down-proj (kxm = SBUF hrs) ----
            # res_out stores go out on the ACT (scalar) HWDGE ring so
            # they never head-of-line block the weight loads streaming on
            # the sync ring.
            res_out_ap = res_out[:].rearrange("(mt ms p) n -> p mt ms n",
                                              p=128, ms=4)
            for m in range(2):
                for nn in range(4):
                    mo = prod_pool.tile([128, 4, 512], bf16, tag="prodr")
                    for mi in range(4):
                        ps2 = psum(mi)
                        for ks in range(2):
                            nc.tensor.matmul(
                                ps2[:, :512],
                                hrs_sb[:, ks,
                                       m * 512 + 128 * mi:
                                       m * 512 + 128 * (mi + 1)],
                                rw2_t[:, 2 * nn + ks:2 * nn + ks + 1, :],
                                start=(ks == 0), stop=(ks == 1))
                        # alternate eviction engine: vector alone can't
                        # quite keep up with res2's 8-MM accumulation
                        # groups, which was pacing the PE
                        if mi % 2 == 0:
                            nc.vector.tensor_copy(out=mo[:, mi, :],
                                                  in_=ps2[:, :512])
                        else:
                            nc.scalar.copy(out=mo[:, mi, :],
                                           in_=ps2[:, :512])
                    # one batched store per group: each dma_start costs
                    # ~0.6us of descriptor generation on its sequencer,
                    # so 8 big stores beat 32 small ones by ~14us of
                    # scalar-engine time.
                    nc.scalar.dma_start(
                        out=res_out_ap[:, m, :, nn * 512:(nn + 1) * 512],
                        in_=mo[:, :, :])

            # ---- mm2: MoE down-proj, hand-rolled with ew2 stationary ----
            # moe_outT[h, t] = sum_f ew2[f, h] * hT[f, t]: streaming the
            # C real tokens; output lands transposed ([H, C]) which the
            # host scatter handles for free. The 16-deep ew2 pool lets the
            # whole of ew2 prefetch during res1/res2 on the sync ring.
            kstep2 = 2 if fp8_down else 1
            pm2 = mybir.MatmulPerfMode.DoubleRow if fp8_down else None
            for mt in range(4):
                t2 = ew2pool.tile([128, 16, 512], hdt, tag="t_ew2")
                nc.sync.dma_start(
                    out=t2[:].rearrange("p (k j) c -> p k (j c)", k=4),
                    in_=ew2[:, mt].rearrange("k p f -> p k f"))
                # mi-major: each 128-row output strip finishes every ~1us,
                # so its PSUM eviction (vector) + store (scalar ring)
                # pipeline under the remaining matmuls instead of bunching
                # up in the kernel tail.
                mo = prod_pool.tile([128, 4, 512], bf16, tag="prodh")
                for mi in range(4):
                    ps2 = psum(mi)
                    for kt in range(4):
                        for ks in range(0, 4, kstep2):
                            nc.tensor.matmul(
                                ps2[:, :NC],
                                t2[:, 4 * kt + ks:4 * kt + ks + kstep2,
                                   128 * mi:128 * (mi + 1)],
                                hT_sb[:, 4 * kt + ks:4 * kt + ks + kstep2,
                                      :NC],
                                start=(kt == 0 and ks == 0),
                                stop=(kt == 3 and ks + kstep2 == 4),
                                perf_mode=pm2)
                    nc.vector.tensor_copy(out=mo[:, mi, :NC],
                                          in_=ps2[:, :NC])
                    dst = moe_out[mt * 512 + mi * 128:
                                  mt * 512 + (mi + 1) * 128, :NC]
                    # stores ride the sync ring, which is idle by now
                    # (all loads issued) — the scalar ring is still
                    # draining res2's stores.
                    nc.sync.dma_start(
                        out=dst.rearrange("(o p) c -> p o c", p=128),
                        in_=mo[:, mi:mi + 1, :NC])

    nc.compile()
    return nc


def _np_softmax(x, axis=-1):
    m = np.max(x, axis=axis, keepdims=True)
    e = np.exp(x - m)
    return e / np.sum(e, axis=axis, keepdims=True)


def _rmsnorm(x, w):
    v = np.mean(np.square(x), axis=-1, keepdims=True)
    return x / np.sqrt(v + EPS) * w


def _tile_pack(W, k_tile, m_tile):
    """[K, M] -> [KT, MT, 128, ksub*m_tile] matching the device producers."""
    K, M = W.shape
    kt, mt, ks = K // k_tile, M // m_tile, k_tile // 128
    return np.ascontiguousarray(
        W.reshape(kt, ks, 128, mt, m_tile)
        .transpose(0, 3, 2, 1, 4)
        .reshape(kt, mt, 128, ks * m_tile))


def kernel(hidden_states, attention_mask, position_ids, wq, wk, wv, wo,
           norm1_w, norm_res_w, res_w1, res_w3, res_w2,
           gate_w, e_w1, e_w3, e_w2):
    global LAST_RESULTS
    f4 = np.float32
    x = np.asarray(hidden_states, f4).reshape(S, H)
    amask = np.asarray(attention_mask).reshape(S)
    pos = np.asarray(position_ids).reshape(S).astype(np.int64)

    # ---- host: attention (tiny vs MoE) ----
    inv_freq = 1.0 / (ROPE_THETA ** (np.arange(0, HD, 2, dtype=f4) / HD))
    t = np.arange(S, dtype=f4)
    freqs = np.outer(t, inv_freq)
    emb = np.concatenate([freqs, freqs], axis=-1)
    sin_t, cos_t = np.sin(emb), np.cos(emb)
    s_ = sin_t[pos].astype(f4)
    c_ = cos_t[pos].astype(f4)

    h = _rmsnorm(x, np.asarray(norm1_w, f4))
    q = (h @ np.asarray(wq, f4)).reshape(S, NH, HD).transpose(1, 0, 2)
    k = (h @ np.asarray(wk, f4)).reshape(S, KVH, HD).transpose(1, 0, 2)
    v = (h @ np.asarray(wv, f4)).reshape(S, KVH, HD).transpose(1, 0, 2)

    def rot(z):
        hh = z.shape[-1] // 2
        return np.concatenate([-z[..., hh:], z[..., :hh]], axis=-1)

    q = q * c_[None] + rot(q) * s_[None]
    k = k * c_[None] + rot(k) * s_[None]
    groups = NH // KVH
    k = np.repeat(k, groups, axis=0)
    v = np.repeat(v, groups, axis=0)
    causal = np.tril(np.ones((S, S), bool))
    mask = causal & (amask > 0)[None, :]
    bias = np.where(mask, f4(0.0), np.finfo(f4).min).astype(f4)
    scores = np.einsum('hqd,hkd->hqk', q, k).astype(f4) * f4(1.0 / np.sqrt(HD))
    scores = scores + bias[None]
    p = _np_softmax(scores, axis=-1).astype(f4)
    attn = np.einsum('hqk,hkd->hqd', p, v).transpose(1, 0, 2).reshape(S, H)
    attn = attn @ np.asarray(wo, f4)
    h1 = x + attn
    hr = _rmsnorm(h1, np.asarray(norm_res_w, f4))

    # ---- host: top-2 routing -> per-expert dispatch ----
    logits = x @ np.asarray(gate_w, f4)
    rw_ = _np_softmax(logits.astype(f4), axis=-1)
    ti = np.argsort(-rw_, axis=-1, kind="stable")[:, :TOPK]
    tw = np.take_along_axis(rw_, ti, axis=-1)
    tw = tw / np.sum(tw, axis=-1, keepdims=True)
    wdense = np.zeros((S, E), f4)
    np.add.at(wdense, (np.arange(S)[:, None], ti), tw)

    idxs = [np.where(wdense[:, c] > 0)[0] for c in range(E)]
    maxc = max(len(ix) for ix in idxs)
    C = max(288, -(-maxc // 32) * 32)   # storage capacity, multiple of 32
    NC = maxc                           # matmul free dim = real max load

    # ---- device: expert-parallel dispatched MoE + sharded residual MLP ----
    key = (C, NC, FP8_UP, FP8_DOWN)
    if key not in _COMPILED:
        _COMPILED[key] = _build_nc(C, NC, FP8_UP, FP8_DOWN)
    nc = _COMPILED[key]

    def b16(a):
        return np.asarray(a, f4).astype(BF16)

    def b8(a, s):
        return np.clip(np.asarray(a, f4) * s, -240.0, 240.0).astype(FP8)

    def pack13(w1, w3, blk=256):
        # [w1_b0|w3_b0|w1_b1|w3_b1|...] in 256-col blocks
        nb = w1.shape[1] // blk
        cols = []
        for m in range(nb):
            cols.append(w1[:, m * blk:(m + 1) * blk])
            cols.append(w3[:, m * blk:(m + 1) * blk])
        return np.concatenate(cols, axis=1)

    xT = np.asarray(x.T, f4)
    hrT_t = _tile_pack(b16(hr.T), 512, 512)
    e_w1 = np.asarray(e_w1, f4)
    e_w3 = np.asarray(e_w3, f4)
    e_w2 = np.asarray(e_w2, f4)
    res_w1 = np.asarray(res_w1, f4)
    res_w3 = np.asarray(res_w3, f4)
    res_w2 = np.asarray(res_w2, f4)

    in_maps = []
    for c in range(N_CORES):
        cs = slice(c * FSH, (c + 1) * FSH)
        ix = idxs[c]
        xdT = np.zeros((H, C), f4)
        xdT[:, :len(ix)] = xT[:, ix]
        wv_c = np.zeros((1, C), f4)
        wv_c[0, :len(ix)] = wdense[ix, c]
        ew13p = pack13(e_w1[c], e_w3[c])
        if FP8_UP:
            xd_dev = b8(xdT, SX)
            ew13_dev = b8(ew13p, SW)
            wv_c = wv_c / (SX * SW)  # fold up-proj descale into routing wt
        else:
            xd_dev = b16(xdT)
            ew13_dev = b16(ew13p)
        if FP8_DOWN:
            wv_c = wv_c * SH_  # h stored as fp8 * SH_
            ew2_dev = b8(e_w2[c], SW)
        else:
            ew2_dev = b16(e_w2[c])
        in_maps.append({
            "xdT": _tile_pack(xd_dev, 512, C),
            "hrT": hrT_t,
            "ew13": _tile_pack(ew13_dev, 512, 512),
            "ew2": _tile_pack(ew2_dev, 512, 512),
            "rw13": _tile_pack(
                b16(np.concatenate([res_w1[:, cs], res_w3[:, cs]], axis=1)),
                512, 512),
            "rw2": _tile_pack(b16(res_w2[cs, :]), 256, 512),
            "wvec": np.ascontiguousarray(wv_c.astype(f4)),
        })

    res = run_bass_kernel_spmd(nc, in_maps, core_ids=list(range(N_CORES)))
    LAST_RESULTS = res

    moe_descale = 1.0 / (SH_ * SW) if FP8_DOWN else 1.0
    out = h1.copy()
    for c in range(N_CORES):
        ix = idxs[c]
        out[ix] += np.asarray(res.results[c]["moe_out"], f4).T[:len(ix)] \
            * moe_descale
        out += np.asarray(res.results[c]["res_out"], f4)
    return out.reshape(B, S, H).astype(np.float32)

